# revision 36
# baseline (speedup 1.0000x reference)
"""Causal self-attention with ALiBi on 8 trn2 cores.

Sharding: data-parallel over batch (2) x tensor-parallel over head groups (4).
Core c handles batch b = c // 4, head group g = c % 4 (heads 4g..4g+3).
Each core computes qT/kT/v projections for its 4 heads, flash-style causal
attention with ALiBi folded into the score matmul via 2 augmented K rows
(k_aug = [iota_j; ones], q_aug = [slope; -slope*i]), and a partial output
projection.  Host sums the 4 partials per batch and adds bo.

v3 changes vs the 181us v2 (trace: PE busy 142us, ACT busy 102us over a
182us span; PE cold at 1.2GHz until 30us; 19us of PE gaps in the tail):
- Score chunks packed in PAIRS into one 2-bank PSUM tile, one exp
  ACTIVATE per pair.  The per-ACTIVATE overhead (~290ns: memory-access
  init + seq dispatch) was 47us of the 102us ACT busy; pairing (with the
  two diagonal chunks column-packed at rearranged offsets) cuts exp to
  80 instructions / ~50us, which un-starves the PE in attn(2)/attn(3)
  where fillers ran out.
- ~10 warmup matmuls on a memset tile right at kernel start: the HAM
  activity monitor only un-throttles the PE clock (1.2 -> 2.4GHz) after
  ~3.4us of sustained busy, and input-load waits kept resetting it, so
  all of proj(0) ran at half clock.  The warmups run while the first
  DMAs stream, so proj(0) starts warm.
- Softmax denominator row->partition broadcast via the gpsimd
  partition_broadcast custom instruction (SBUF->SBUF, idle engine)
  instead of the v2 SBUF->DRAM->broadcast DMA round trip, which put
  ~4us on the critical tail of every (qb,h) chain and re-throttled the
  PE clock during the final output projection.
- Startup loads spread across THREE ~150GB/s DMA queues (x0 split by
  partition halves on sync+scalar, wv on vector) with the thr-tile
  prefetch throttle reinstated so wo/x2/x3 can't steal bandwidth from
  the startup-critical loads (v3 measured x0 landing at 15.4us because
  everything loaded concurrently).
- qk projection units emitted in two halves for finer weave grain.
- x / out DRAM tensors are per-512-block contiguous ([4,P,8,512]) so
  each block transfers as 128 fat descriptors instead of 1024 thin
  ones; load queues rebalanced (sync: x0/wq/wk/x1, scalar: wv/wo/masks,
  gpsimd: aug rows + x2/x3) so the vector engine is free for compute.
"""

import sys

sys.path.insert(0, "/opt/trn_rl_repo")

import numpy as np

import concourse.bacc as bacc
import concourse.mybir as mybir
import concourse.tile as tile
from concourse.bass import ds, ts
from concourse.bass_utils import run_bass_kernel_spmd

B, T, D, H, DH = 2, 2048, 1024, 16, 64
G = 4            # head groups (tensor-parallel)
HPC = H // G     # heads per core
DG = D // G      # model dims per core (256)
P = 128
N_CORES = 8
NEG = -1.0e30

F32 = mybir.dt.float32
F32R = mybir.dt.float32r
BF16 = mybir.dt.bfloat16
ADD = mybir.AluOpType.add
MULT = mybir.AluOpType.mult
EXP = mybir.ActivationFunctionType.Exp

TRACE = False
DEBUG = False
LAST_RESULTS = None

_cache = {}


def _build(with_bias: bool):
    nc = bacc.Bacc("TRN2", target_bir_lowering=False, debug=False)

    xT_d = nc.dram_tensor("xT", [4, P, 8, 512], BF16, kind="ExternalInput").ap()
    wq_d = nc.dram_tensor("wqT", [P, 8, DG], BF16, kind="ExternalInput").ap()
    wk_d = nc.dram_tensor("wkT", [P, 8, DG], BF16, kind="ExternalInput").ap()
    wv_d = nc.dram_tensor("wvT", [P, 8, DG], BF16, kind="ExternalInput").ap()
    wo_d = nc.dram_tensor("woT", [P, 2, D], BF16, kind="ExternalInput").ap()
    qaug_d = nc.dram_tensor("qaug", [HPC, 2, T], F32, kind="ExternalInput").ap()
    kaug_d = nc.dram_tensor("kaug", [2, T], F32, kind="ExternalInput").ap()
    # ident + causal-stair mask packed in one tensor: 512B/partition lines
    # DMA twice as fast as two separate 256B/partition transfers
    masks_d = nc.dram_tensor("masks", [P, 2, P], BF16, kind="ExternalInput").ap()
    if with_bias:
        bvo_d = nc.dram_tensor("bvo", [P, DG], F32, kind="ExternalInput").ap()
        bq_d = nc.dram_tensor("bq2", [P, 2], F32, kind="ExternalInput").ap()
        bk_d = nc.dram_tensor("bk2", [P, 2], F32, kind="ExternalInput").ap()
    out_d = nc.dram_tensor("outT", [4, P, 8, 512], BF16, kind="ExternalOutput").ap()

    with tile.TileContext(nc) as tc:
        with (
            tc.tile_pool(name="big", bufs=1) as big,
            tc.tile_pool(name="stage", bufs=3) as stage,
            tc.tile_pool(name="obp", bufs=2) as obp,
            tc.tile_pool(name="expp", bufs=4) as expp,
            tc.tile_pool(name="small", bufs=3) as small,
            tc.tile_pool(name="pj", bufs=2, space="PSUM") as pjp,
            tc.tile_pool(name="pss", bufs=2, space="PSUM") as pssp,
            tc.tile_pool(name="psy", bufs=2, space="PSUM") as psyp,
        ):
            # ---- persistent tiles
            xb = [
                big.tile([P, 8, 512], BF16, tag=f"x{tq}", name=f"x{tq}")
                for tq in range(4)
            ]
            wv = big.tile([P, 8, DG], BF16, tag="wv", name="wv")
            wq = big.tile([P, 8, DG], BF16, tag="wq", name="wq")
            wk = big.tile([P, 8, DG], BF16, tag="wk", name="wk")
            wo = big.tile([P, 2, D], BF16, tag="wo", name="wo")
            qa = [big.tile([66, T], F32R, tag=f"qa{h}", name=f"qa{h}") for h in range(HPC)]
            ka = [big.tile([66, T], F32R, tag=f"ka{h}", name=f"ka{h}") for h in range(HPC)]
            va = [big.tile([P, 16, P], BF16, tag=f"va{h}", name=f"va{h}") for h in range(HPC)]
            yt = [big.tile([P, T], BF16, tag=f"yt{m}", name=f"yt{m}") for m in range(2)]

            # ---- PE warmup: the HAM clock gate holds the PE at 1.2GHz until
            # it has seen ~3.4us of sustained matmul activity.  Run throwaway
            # matmuls on a memset tile while the input DMAs stream so the
            # real projections start at 2.4GHz.
            # 30 warmup matmuls bridge the ~9us from queue-start to x0+wv
            # landing (~8 run cold at 426ns flipping the HAM, the rest warm)
            warm = big.tile([P, 512], BF16, tag="warm", name="warm")
            nc.vector.memset(warm[:], 0.0)
            wps = pjp.tile([P, 512], F32, tag="pj", name="wps")
            for i in range(30):
                nc.tensor.matmul(
                    out=wps[:],
                    lhsT=warm[:, 0:P],
                    rhs=warm[:],
                    start=(i == 0),
                    stop=(i == 29),
                )

            # ---- loads.  Queue->DGE mapping measured from traces: sync and
            # scalar drive HARDWARE DGE queues whose packets start at ~9 and
            # ~11us; the gpsimd queue is SOFTWARE DGE starting ~13us.  Each
            # queue tops out at ~120-160GB/s.  The first-needed tensors (x0
            # whole -- a reader of any slice waits on ALL DMAs into the
            # tile, so no column/kc splits -- then wv/wq) go on the two HW
            # queues in need-order; wk + aug rows + the x1/wo/x2/x3
            # prefetches ride gpsimd/sync behind them, naturally throttled
            # by queue order.
            nc.sync.dma_start(out=xb[0][:], in_=xT_d[0])
            nc.scalar.dma_start(out=wv[:], in_=wv_d[:])
            nc.scalar.dma_start(out=wq[:], in_=wq_d[:])
            nc.sync.dma_start(out=wk[:], in_=wk_d[:])
            masks_sb = big.tile([P, 2, P], BF16, tag="masks")
            nc.scalar.dma_start(out=masks_sb[:], in_=masks_d[:])
            ident_sb = masks_sb[:, 0, :]
            maskst_sb = masks_sb[:, 1, :]
            for h in range(HPC):
                nc.gpsimd.dma_start(out=qa[h][64:66, :], in_=qaug_d[h].bitcast(F32R))
                nc.gpsimd.dma_start(out=ka[h][64:66, :], in_=kaug_d[:].bitcast(F32R))
            nc.gpsimd.dma_start(out=xb[1][:], in_=xT_d[1])
            nc.gpsimd.dma_start(out=wo[:], in_=wo_d[:])
            nc.gpsimd.dma_start(out=xb[2][:], in_=xT_d[2])
            nc.gpsimd.dma_start(out=xb[3][:], in_=xT_d[3])
            if with_bias:
                bvo = big.tile([P, DG], F32, tag="bvo")
                nc.sync.dma_start(out=bvo[:], in_=bvo_d[:])
                bq2 = big.tile([P, 2], F32, tag="bq2")
                nc.sync.dma_start(out=bq2[:], in_=bq_d[:])
                bk2 = big.tile([P, 2], F32, tag="bk2")
                nc.sync.dma_start(out=bk2[:], in_=bk_d[:])
            for h in range(HPC):
                # ones column for the in-matmul softmax denominator.  Odd
                # heads (v dims at py rows 64:128) put it at column 0 so the
                # denominator lands at py row 0, where partition_broadcast
                # (which requires base-partition-0 operands) can read it
                # directly; even heads' lands at row 64 and needs a row-move.
                oc = 64 if h % 2 == 0 else 0
                for ch in range(16):
                    nc.vector.memset(va[h][:, ch, oc : oc + 1], 1.0)


            # ---- emission units.  Each is (pe_cost_estimate, closure); the
            # weave below merges the attention backbone with independent
            # filler matmuls so the in-order PE stream never sits on an
            # exp-wait (stalls also drop the p-state to 1.2GHz).
            def unit_v(tq, ch):
                def emit():
                    lc = (ch % 4) * P
                    pv = pjp.tile([P, DG], F32, tag="pj", name=f"pv{4 * tq + ch}")
                    for kc in range(8):
                        nc.tensor.matmul(
                            out=pv[:],
                            lhsT=xb[tq][:, kc, lc : lc + P],
                            rhs=wv[:, kc, :],
                            start=(kc == 0),
                            stop=(kc == 7),
                        )
                    chg = 4 * tq + ch
                    for h in range(HPC):
                        off = 0 if h % 2 == 0 else 64
                        if with_bias:
                            nc.vector.tensor_tensor(
                                out=va[h][:, chg, off : off + 64],
                                in0=pv[:, h * 64 : h * 64 + 64],
                                in1=bvo[:, h * 64 : h * 64 + 64],
                                op=ADD,
                            )
                        else:
                            nc.vector.tensor_copy(
                                out=va[h][:, chg, off : off + 64],
                                in_=pv[:, h * 64 : h * 64 + 64],
                            )

                return (860, emit)

            # each qk projection is emitted as TWO units (kc 0-3, kc 4-7)
            # so the weave can interleave backbone work at ~0.9us grain
            def unit_qk(tq, which, mc):
                pq_box = {}

                def emit_a():
                    wt = wq if which == "q" else wk
                    pq_box["pq"] = pjp.tile(
                        [P, 512], F32, tag="pj", name=f"p{which}{tq}_{mc}"
                    )
                    for kc in range(4):
                        nc.tensor.matmul(
                            out=pq_box["pq"][:],
                            lhsT=wt[:, kc, ds(mc * P, P)],
                            rhs=xb[tq][:, kc, :],
                            start=(kc == 0),
                            stop=False,
                        )

                def emit_b():
                    wt, dst = (wq, qa) if which == "q" else (wk, ka)
                    bt = None
                    if with_bias:
                        bt = bq2 if which == "q" else bk2
                    pq = pq_box["pq"]
                    for kc in range(4, 8):
                        nc.tensor.matmul(
                            out=pq[:],
                            lhsT=wt[:, kc, ds(mc * P, P)],
                            rhs=xb[tq][:, kc, :],
                            start=False,
                            stop=(kc == 7),
                        )
                    h_even, h_odd = 2 * mc, 2 * mc + 1
                    if with_bias:
                        nc.vector.tensor_scalar(
                            out=dst[h_even][0:64, ts(tq, 512)],
                            in0=pq[0:64, :],
                            scalar1=bt[0:64, mc : mc + 1],
                            scalar2=None,
                            op0=ADD,
                        )
                    else:
                        nc.vector.tensor_copy(
                            out=dst[h_even][0:64, ts(tq, 512)], in_=pq[0:64, :]
                        )
                    st = stage.tile([P, 512], F32R, tag="stage", name="st")
                    if with_bias:
                        nc.vector.tensor_scalar(
                            out=st[64:128, :],
                            in0=pq[64:128, :],
                            scalar1=bt[64:128, mc : mc + 1],
                            scalar2=None,
                            op0=ADD,
                        )
                    else:
                        nc.vector.tensor_copy(out=st[64:128, :], in_=pq[64:128, :])
                    nc.sync.dma_start(
                        out=dst[h_odd][0:64, ts(tq, 512)], in_=st[64:128, :]
                    )

                return [(875, emit_a), (875, emit_b)]

            obs_tiles = {}

            # final=True (the out(3) units after the last backbone): po
            # tiles alternate pj/psy PSUM pools (psy is free then) to break
            # the 2-slot WAR serialization, casts alternate DVE/ACT, and
            # the store splits per 2 ec so transfers overlap the casts.
            def unit_outproj(qb, ec, final=False):
                def emit():
                    if ec == 0:
                        obs_tiles[qb] = obp.tile(
                            [P, 8, 512], BF16, tag="obs", name=f"obs{qb}"
                        )
                    obs = obs_tiles[qb]
                    pool = psyp if final and ec % 2 else pjp
                    tag = "psy" if final and ec % 2 else "pj"
                    po = pool.tile([P, 512], F32, tag=tag, name=f"po{qb}_{ec}")
                    for k2 in range(2):
                        nc.tensor.matmul(
                            out=po[:, 0:512],
                            lhsT=wo[:, k2, ds(ec * P, P)],
                            rhs=yt[k2][:, ts(qb, 512)],
                            start=(k2 == 0),
                            stop=(k2 == 1),
                        )
                    if final and ec % 2:
                        nc.scalar.copy(out=obs[:, ec, :], in_=po[:, 0:512])
                    else:
                        nc.vector.tensor_copy(out=obs[:, ec, :], in_=po[:, 0:512])
                    if final:
                        nc.sync.dma_start(
                            out=out_d[qb][:, ec : ec + 1, :],
                            in_=obs[:, ec : ec + 1, :],
                        )
                    elif ec == 3:
                        nc.sync.dma_start(
                            out=out_d[qb][:, 0:4, :], in_=obs[:, 0:4, :]
                        )
                    elif ec == 7:
                        nc.sync.dma_start(
                            out=out_d[qb][:, 4:8, :], in_=obs[:, 4:8, :]
                        )

                return (500, emit)

            # attention backbone for one (qb, h): score chunks are emitted in
            # PAIRS packed into one 2-bank PSUM tile so a single exp ACTIVATE
            # covers both (halving ACT instruction overhead).  The diagonal
            # chunks are column-packed at rearranged offsets: their AV matmuls
            # read the packed slice and write the true query window of py.
            # AV runs one pair behind exp, then the denominator/normalize
            # epilogue (reciprocal -> K=1 broadcast matmul -> multiply).
            def attn_units(qb, h):
                o = qb * 512
                state = {}

                # groups of (jc, lo, off): lo = query offset into the 512
                # block (width 512-lo), off = column offset in the pair tile
                groups = []
                for j0 in range(0, 4 * qb, 2):
                    groups.append([(j0, 0, 0), (j0 + 1, 0, 512)])
                dj = 4 * qb
                groups.append([(dj, 0, 0), (dj + 1, 128, 512)])
                groups.append([(dj + 2, 256, 0), (dj + 3, 384, 256)])

                def mk_group(gi, chunks):
                    wtot = sum(512 - lo for _, lo, _ in chunks)

                    def emit():
                        if gi == 0:
                            state["py"] = psyp.tile(
                                [P, 512], F32, tag="psy", name=f"py{qb}_{h}"
                            )
                            state["pend"] = []
                        py = state["py"]
                        ps = pssp.tile(
                            [P, 1024], F32, tag="pss", name=f"ps{qb}_{h}_{gi}"
                        )
                        for jc, lo, off in chunks:
                            w = 512 - lo
                            diag = jc * 128 - o >= 0
                            nc.tensor.matmul(
                                out=ps[:, off : off + w],
                                lhsT=ka[h][0:66, ts(jc, P)],
                                rhs=qa[h][0:66, ds(o + lo, w)],
                                start=True,
                                stop=not diag,
                            )
                            if diag:
                                # causal stair: ps[:, off:off+128] += I.T @ maskst
                                nc.tensor.matmul(
                                    out=ps[:, off : off + P],
                                    lhsT=ident_sb[:],
                                    rhs=maskst_sb[:],
                                    start=False,
                                    stop=True,
                                )
                        ex = expp.tile(
                            [P, 1024], BF16, tag="ex", name=f"ex{qb}_{h}_{gi}"
                        )
                        nc.scalar.activation(
                            out=ex[:, 0:wtot], in_=ps[:, 0:wtot], func=EXP
                        )
                        # AV runs TWO groups behind exp: the next group's
                        # score matmuls (~430ns) alone don't cover the ~1us
                        # exp, so a 1-group lag stalls the in-order PE on
                        # the exp semaphore (v3 trace: ~1us waits per group)
                        pend = state["pend"]
                        pend.append([(jc, lo, off, ex) for jc, lo, off in chunks])
                        if len(pend) > 2:
                            for pjc, plo, poff, pex in pend.pop(0):
                                nc.tensor.matmul(
                                    out=py[:, plo:512],
                                    lhsT=va[h][:, pjc, :],
                                    rhs=pex[:, poff : poff + 512 - plo],
                                    start=(pjc == 0),
                                    stop=False,
                                )

                    ndiag = sum(1 for jc, _, _ in chunks if jc * 128 - o >= 0)
                    cost = wtot + 128 * ndiag + (1024 if gi > 1 else 0)
                    return (int(cost * 0.42), emit)

                def emit_tail():
                    py = state["py"]
                    groups_left = state.pop("pend")
                    flat = [c for grp in groups_left for c in grp]
                    for i, (pjc, plo, poff, pex) in enumerate(flat):
                        nc.tensor.matmul(
                            out=py[:, plo:512],
                            lhsT=va[h][:, pjc, :],
                            rhs=pex[:, poff : poff + 512 - plo],
                            start=(pjc == 0),
                            stop=(i == len(flat) - 1),
                        )

                def emit_norm():
                    py = state["py"]
                    rowbase = (h % 2) * 64
                    dn = small.tile([P, 512], F32, tag="dn", name=f"dn{qb}_{h}")
                    # approx recip mis-executes on partition-base slices;
                    # full-tile costs the same (DVE time ~ free size only).
                    # Junk rows of py produce junk reciprocals, never read.
                    nc.vector.reciprocal_approx_fast(out=dn[:], in_=py[:])
                    # row->partition broadcast on the (idle) gpsimd engine:
                    # SBUF->SBUF, no DMA round trip through DRAM.  HW
                    # partition_broadcast requires BOTH operands at base
                    # partition 0 (nonzero bases read/write garbage --
                    # measured), so odd heads (denominator at py row 0 via
                    # the ones column at va column 0) broadcast directly;
                    # even heads (denominator at row 64) first move that row
                    # to partition 0 with a tiny sync-queue DMA.
                    if h % 2 == 0:
                        src = small.tile([P, 512], F32, tag="dn2", name=f"dn2{qb}_{h}")
                        nc.sync.dma_start(out=src[0:1, :], in_=dn[64:65, :])
                    else:
                        src = dn
                    rb = small.tile([P, 512], F32, tag="rb", name=f"rb{qb}_{h}")
                    nc.gpsimd.partition_broadcast(
                        out_ap=rb[:, :], in_ap=src[0:1, :]
                    )
                    nc.vector.tensor_tensor(
                        out=yt[h // 2][rowbase : rowbase + 64, ds(o, 512)],
                        in0=py[rowbase : rowbase + 64, :],
                        in1=rb[rowbase : rowbase + 64, :],
                        op=MULT,
                    )

                units = [mk_group(gi, chunks) for gi, chunks in enumerate(groups)]
                units.append((int(1280 * 0.42), emit_tail))
                units.append((300, emit_norm))
                return units

            # fillers run AHEAD of the backbone by FRONT: the next phase's
            # backbone depends on this phase's proj fillers through DVE-copy
            # and DMA-shift chains, so fillers finishing flush with the
            # phase end stall the next phase's first score matmuls
            # (v5 trace: 3.7us hole at the attn(0)->attn(1) boundary)
            # dependency-free matmuls on the warm tile: fill known PE-stall
            # windows (phase boundaries, the final norm chain) so the HAM
            # activity monitor never sees an idle window and re-throttles
            # the clock to 1.2GHz.  Writes go to a scratch pss slot; the
            # in-order PE runs them while the real successors wait on sems.
            def unit_dummy(n, tag):
                def emit():
                    scr = pssp.tile([P, 1024], F32, tag="pss", name=f"dm{tag}")
                    for i in range(n):
                        nc.tensor.matmul(
                            out=scr[:, 0:512],
                            lhsT=warm[:, 0:P],
                            rhs=warm[:],
                            start=(i == 0),
                            stop=(i == n - 1),
                        )

                return (int(n * 512 * 0.42), emit)

            def weave(backbone, fillers, front=2.0):
                tb = sum(c for c, _ in backbone) or 1
                tf = sum(c for c, _ in fillers) or 1
                ib = jf = 0
                cb = cf = 0.0
                while ib < len(backbone) or jf < len(fillers):
                    take_b = jf >= len(fillers) or (
                        ib < len(backbone) and front * cb / tb <= cf / tf
                    )
                    if take_b:
                        c, fn = backbone[ib]
                        ib += 1
                        cb += c
                    else:
                        c, fn = fillers[jf]
                        jf += 1
                        cf += c
                    fn()

            def proj_units(tq):
                # tq=0: v first (x0 lands before wq/wk).  q before k (wq
                # loads before wk).  tq>=1: q/k first so the next attention
                # block's score->exp chain starts earlier (v is only needed
                # one chunk later, by the first AV).
                vs = [unit_v(tq, ch) for ch in range(4)]
                qks = []
                for mc in range(2):
                    qks += unit_qk(tq, "q", mc)
                for mc in range(2):
                    qks += unit_qk(tq, "k", mc)
                return vs + qks if tq == 0 else qks + vs

            # ---- schedule: proj(0) | attn(0)+proj(1) | attn(1)+proj(2)+out(0)
            #      | attn(2)+proj(3)+out(1) | attn(3)+out(2) | out(3)
            for _, fn in proj_units(0):
                fn()
            for t in range(1, 4):
                backbone = []
                for h in range(HPC):
                    backbone += attn_units(t - 1, h)
                fillers = [unit_dummy(6, f"t{t}")] if t == 2 else []
                fillers += proj_units(t)
                if t >= 2:
                    fillers += [unit_outproj(t - 2, ec) for ec in range(8)]
                weave(backbone, fillers)
            backbone = []
            for h in range(HPC):
                backbone += attn_units(3, h)
            weave(backbone, [unit_outproj(2, ec) for ec in range(8)])
            # bridge the final norm chain (recip->broadcast->mult, ~3us with
            # no remaining backbone) so outproj(3) starts at full clock
            unit_dummy(12, "tail")[1]()
            for ec in range(8):
                unit_outproj(3, ec, final=True)[1]()

    nc.compile()
    return nc


def _get_nc(with_bias: bool):
    key = (with_bias, DEBUG)
    if key not in _cache:
        _cache[key] = _build(with_bias)
    return _cache[key]


def kernel(x, freqs_cis, Wq, bq, Wkv, bkv, Wo, bo, **_unused):
    import ml_dtypes

    bf16 = ml_dtypes.bfloat16

    x = np.asarray(x, np.float32)
    Wq = np.asarray(Wq, np.float32)
    bq = np.asarray(bq, np.float32)
    Wkv = np.asarray(Wkv, np.float32)
    bkv = np.asarray(bkv, np.float32)
    Wo = np.asarray(Wo, np.float32)
    bo = np.asarray(bo, np.float32)

    with_bias = bool(np.any(bq) or np.any(bkv))
    nc = _get_nc(with_bias)

    scale = 1.0 / np.sqrt(DH)
    iota = np.arange(T, dtype=np.float32)

    # causal stair (applied via identity-matmul accumulation into PSUM):
    # maskst[p, m] = -1e30 where m < p (j = chunk base + p is in the future)
    mm = np.arange(P, dtype=np.float32)
    maskst = np.where(mm[None, :] < mm[:, None], NEG, 0.0).astype(bf16)
    ident = np.eye(P, dtype=bf16)
    masks = np.ascontiguousarray(np.stack([ident, maskst], axis=1))  # [P,2,P]

    kaug = np.stack([iota, np.ones(T, np.float32)])  # [2, T]

    # p-major packing, per-512-block contiguous: x -> [tq, p, kc, tlocal]
    # so each block (and each column-half of block 0) loads with one DMA
    # of 128 fat descriptors
    xT = [
        np.ascontiguousarray(
            x[b].T.reshape(8, P, 4, 512).transpose(2, 1, 0, 3)
        ).astype(bf16)
        for b in range(B)
    ]

    in_maps = []
    for c in range(N_CORES):
        b, g = divmod(c, G)
        rows = slice(g * DG, (g + 1) * DG)
        def pack(wT, n):  # [n*P, cols] -> [P, n, cols]
            return np.ascontiguousarray(
                wT.reshape(n, P, wT.shape[1]).transpose(1, 0, 2)
            ).astype(bf16)

        wqT = pack((Wq[rows] * scale).T, 8)
        wkT = pack(Wkv[0:D][rows].T, 8)
        wvT = pack(Wkv[D : 2 * D][rows].T, 8)
        woT = pack(Wo[:, rows].T, 2)
        qaug = np.zeros((HPC, 2, T), np.float32)
        for h in range(HPC):
            slope = (g * HPC + h + 1) / H
            qaug[h, 0, :] = slope
            qaug[h, 1, :] = -slope * iota
        m = {
            "xT": xT[b],
            "wqT": wqT,
            "wkT": wkT,
            "wvT": wvT,
            "woT": woT,
            "qaug": qaug,
            "kaug": kaug,
            "masks": masks,
        }
        if with_bias:
            bv_g = bkv[D : 2 * D][rows]
            m["bvo"] = np.ascontiguousarray(
                np.broadcast_to(bv_g[None, :], (P, DG))
            ).astype(np.float32)
            m["bq2"] = np.ascontiguousarray((bq[rows] * scale).reshape(2, P).T)
            m["bk2"] = np.ascontiguousarray(bkv[0:D][rows].reshape(2, P).T)
        in_maps.append(m)

    res = run_bass_kernel_spmd(nc, in_maps, list(range(N_CORES)), trace=TRACE)
    global LAST_RESULTS
    LAST_RESULTS = res

    out = np.empty((B, T, D), np.float32)
    for b in range(B):
        acc = res.results[b * G]["outT"].astype(np.float32)  # [4, P, 8, 512]
        for g in range(1, G):
            acc += res.results[b * G + g]["outT"].astype(np.float32)
        # [tq, p, ec, t] -> per block [ec*128+p, 512] -> [T, D]
        out[b] = np.concatenate(
            [acc[tq].transpose(1, 0, 2).reshape(D, 512).T for tq in range(4)],
            axis=0,
        ) + bo[None, :]
    return out
